# revision 4
# baseline (speedup 1.0000x reference)
"""DMAGLSTMCell Trainium2 kernel — data-parallel over batch on 8 NeuronCores.

Design (per core, batch shard of 8 rows):
  - All weights live in SBUF as bf16, packed for the PE stationary operand:
    Wsb[p, kc*2816 + mt*128 + c] = W_all[kc*128 + p, mt*128 + c] where
    W_all = [W_f_short | W_f_long | W_alpha | W_o | W_m | W_C]  (768 x 2816).
  - Activations flow transposed: PSUM [gate-dim-tile on partitions, batch on
    free], packed 22 m-tiles x 8 batch = [128, 176] in one PSUM bank.
  - Phase A precomputes the x-contribution gx[t] = x_t @ W_x + b for all t
    (parallel over T) into DRAM; the sequential loop adds it back per step
    with a single identity-matmul (PE accumulates it into PSUM directly).
  - Recurrence: For_i over T in strides of 8 (8 steps unrolled in the body),
    double-buffered gx prefetch DMA, h history kept in SBUF (bf16) and the
    next step's matmul rhs reads the history slice directly.
"""
import sys
sys.path.insert(0, "/opt/trn_rl_repo")

import numpy as np
import ml_dtypes

BF16 = ml_dtypes.bfloat16

B, T, D, U = 64, 512, 256, 512
NC = 8            # cores
BS = B // NC      # batch shard per core = 8
KH = U // 128     # h-part contraction chunks = 4
KX = D // 128     # x-part contraction chunks = 2
MT_G = (4 * U + D) // 128   # gate m-tiles (fs,fl,alpha,o,m) = 18
MT_C = U // 128             # c-bar m-tiles = 4
MT = MT_G + MT_C            # 22
GF = MT_G * BS              # gates psum free width = 144
PF = MT * BS                # full psum free width = 176
WCOL = 2816                 # total output columns
TB = 64                     # phase-A t-block
NTB = T // TB               # 8
STG = TB * PF               # stage free size (gx slot incl b_C tail)
UNROLL = 8

_CACHE = {}


def _build_program(t_steps):
    import concourse.bass as bass
    import concourse.bacc as bacc
    import concourse.mybir as mybir
    from concourse import tile
    from concourse.bass import ds

    f32 = mybir.dt.float32
    bf16 = mybir.dt.bfloat16
    AF = mybir.ActivationFunctionType

    ntb = t_steps // TB
    nc = bacc.Bacc("TRN2", target_bir_lowering=False)

    # ---- DRAM I/O ----
    wsb_d = nc.dram_tensor("wsb", [128, 6 * WCOL], bf16, kind="ExternalInput")
    xt_d = nc.dram_tensor("xt", [128, KX * t_steps * BS], bf16, kind="ExternalInput")
    b22_d = nc.dram_tensor("b22", [128, MT], f32, kind="ExternalInput")
    bc64_d = nc.dram_tensor("bc64", [128, TB * MT_C * BS], bf16,
                            kind="ExternalInput")
    h0_d = nc.dram_tensor("h0p", [128, KH * BS], bf16, kind="ExternalInput")
    c0_d = nc.dram_tensor("c0p", [128, MT_C * BS], f32, kind="ExternalInput")
    eye_d = nc.dram_tensor("eye", [128, 128], bf16, kind="ExternalInput")
    ho_d = nc.dram_tensor("ho", [128, t_steps * KH * BS], f32, kind="ExternalOutput")
    gx_d = nc.dram_tensor("gxd", [128, t_steps * PF + 2 * UNROLL * PF], bf16,
                          kind="Internal")

    with tile.TileContext(nc) as tc:
        with (
            tc.tile_pool(name="persist", bufs=1) as pp,
            tc.tile_pool(name="stage", bufs=2) as sp,
            tc.tile_pool(name="scratch", bufs=2) as scp,
            tc.tile_pool(name="psA", bufs=2, space="PSUM") as ppA,
            tc.tile_pool(name="psM", bufs=2, space="PSUM") as ppM,
        ):
            # ---- persistent SBUF ----
            wsb = pp.tile([128, 6 * WCOL], bf16)
            xt = pp.tile([128, KX * t_steps * BS], bf16)
            b22 = pp.tile([128, MT], f32)
            eye = pp.tile([128, 128], bf16)
            hist = pp.tile([128, (t_steps + 1) * KH * BS], bf16)
            cbuf = [pp.tile([128, MT_C * BS], f32, name=f"cst{i}", tag=f"c{i}")
                    for i in range(2)]
            gxb = [pp.tile([128, UNROLL // 2 * PF], bf16, name=f"gxb{i}",
                           tag=f"gx{i}") for i in range(2)]

            nc.sync.dma_start(wsb[:], wsb_d[:])
            nc.sync.dma_start(xt[:], xt_d[:])
            nc.sync.dma_start(b22[:], b22_d[:])
            nc.sync.dma_start(eye[:], eye_d[:])
            nc.sync.dma_start(hist[:, 0:KH * BS], h0_d[:])
            nc.sync.dma_start(cbuf[0][:], c0_d[:])

            def w_ap(kc, mt, ncols=128):
                return wsb[:, kc * WCOL + mt * 128: kc * WCOL + mt * 128 + ncols]

            # ---- Phase A: gx[t] = x_t @ W_x + b_gates for all t ----
            for tb in range(ntb):
                stage = sp.tile([128, STG], bf16, tag="stage")
                st3 = stage[:].rearrange("p (t m) -> p t m", t=TB)
                for mt in range(MT_G):
                    ps = ppA.tile([128, TB * BS], f32, tag="psA")
                    for kc in range(KX):
                        rhs = xt[:, kc * t_steps * BS + tb * TB * BS:
                                 kc * t_steps * BS + (tb + 1) * TB * BS]
                        nc.tensor.matmul(ps[:], w_ap(4 + kc, mt), rhs,
                                         start=(kc == 0), stop=(kc == KX - 1))
                    ps3 = ps[:].rearrange("p (t b) -> p t b", t=TB)
                    nc.vector.tensor_scalar_add(
                        st3[:, :, mt * BS:(mt + 1) * BS], ps3, b22[:, mt:mt + 1])
                nc.sync.dma_start(
                    st3[:, :, GF:PF], bc64_d[:].rearrange(
                        "p (t m) -> p t m", t=TB))
                nc.sync.dma_start(gx_d[:, tb * STG:(tb + 1) * STG], stage[:])

            # zero the prefetch-overrun pad past the last real gx column
            zpad = pp.tile([128, 2 * UNROLL * PF], bf16)
            nc.vector.memset(zpad[:], 0.0)
            nc.sync.dma_start(
                gx_d[:, t_steps * PF:t_steps * PF + 2 * UNROLL * PF], zpad[:])

            # preload first two gx buffers (steps 0-3 / 4-7)
            half = UNROLL // 2 * PF
            nc.sync.dma_start(gxb[0][:], gx_d[:, 0:half])
            nc.sync.dma_start(gxb[1][:], gx_d[:, half:2 * half])

            # ---- recurrence ----
            with tc.For_i(0, t_steps, UNROLL) as iv:
                for u in range(UNROLL):
                    buf = gxb[(u // 4) % 2]
                    ui = u % 4
                    cprev = cbuf[u % 2]
                    cnew = cbuf[(u + 1) % 2]
                    ps = ppM.tile([128, PF], f32, tag="psM")

                    # gx + gate bias + c-bar bias via one identity matmul
                    nc.tensor.matmul(ps[:], eye[:],
                                     buf[:, ui * PF:(ui + 1) * PF],
                                     start=True, stop=False, skip_group_check=True)
                    # h-part sweep: gates then c-bar tiles
                    for mt in range(MT_G):
                        for kc in range(KH):
                            rhs = hist[:, ds((iv + u) * KH * BS + kc * BS, BS)]
                            nc.tensor.matmul(ps[:, mt * BS:(mt + 1) * BS],
                                             w_ap(kc, mt), rhs,
                                             start=False, stop=(kc == KH - 1),
                                             skip_group_check=True)
                    for mt in range(MT_G, MT):
                        for kc in range(KH):
                            rhs = hist[:, ds((iv + u) * KH * BS + kc * BS, BS)]
                            nc.tensor.matmul(ps[:, mt * BS:(mt + 1) * BS],
                                             w_ap(kc, mt), rhs,
                                             start=False, stop=False,
                                             skip_group_check=True)

                    # sigmoid over gates+m
                    G = scp.tile([128, GF], bf16, tag="G")
                    nc.scalar.activation(G[:], ps[:, 0:GF], AF.Sigmoid)

                    # modx = m * x_t   (two halves of D)
                    modx = scp.tile([128, KX * BS], bf16, tag="modx")
                    for kc in range(KX):
                        nc.vector.tensor_mul(
                            modx[:, kc * BS:(kc + 1) * BS],
                            G[:, (16 + kc) * BS:(17 + kc) * BS],
                            xt[:, ds(kc * t_steps * BS + (iv + u) * BS, BS)])
                    # MM3: c-bar += modx @ W_C_x
                    for mt in range(MT_G, MT):
                        for kc in range(KX):
                            nc.tensor.matmul(ps[:, mt * BS:(mt + 1) * BS],
                                             w_ap(4 + kc, mt),
                                             modx[:, kc * BS:(kc + 1) * BS],
                                             start=False, stop=(kc == KX - 1),
                                             skip_group_check=True)

                    # f = fl + alpha*(fs - fl)
                    uu = scp.tile([128, MT_C * BS], bf16, tag="uu")
                    ww = scp.tile([128, MT_C * BS], bf16, tag="ww")
                    ff = scp.tile([128, MT_C * BS], f32, tag="ff")
                    nc.vector.tensor_sub(uu[:], G[:, 0:32], G[:, 32:64])
                    nc.vector.tensor_mul(ww[:], G[:, 64:96], uu[:])
                    nc.vector.tensor_add(ff[:], G[:, 32:64], ww[:])

                    # c-bar = tanh(psum tail)
                    cbar = scp.tile([128, MT_C * BS], f32, tag="cbar")
                    nc.scalar.activation(cbar[:], ps[:, GF:PF], AF.Tanh)

                    # c update: c = cbar + f*(c_prev - cbar)
                    dd = scp.tile([128, MT_C * BS], f32, tag="dd")
                    ee = scp.tile([128, MT_C * BS], f32, tag="ee")
                    nc.vector.tensor_sub(dd[:], cprev[:], cbar[:])
                    nc.vector.tensor_mul(ee[:], ff[:], dd[:])
                    nc.vector.tensor_add(cnew[:], ee[:], cbar[:])

                    # h = o * tanh(c) -> straight into history
                    th = scp.tile([128, MT_C * BS], bf16, tag="th")
                    nc.scalar.activation(th[:], cnew[:], AF.Tanh)
                    nc.vector.tensor_mul(
                        hist[:, ds((iv + u + 1) * KH * BS, KH * BS)],
                        G[:, 96:128], th[:])

                    # refill the gx half-buffer we just finished with
                    if u == 3:
                        nc.sync.dma_start(
                            gxb[0][:], gx_d[:, ds((iv + UNROLL) * PF, half)])
                    if u == 7:
                        nc.sync.dma_start(
                            gxb[1][:], gx_d[:, ds((iv + UNROLL + 4) * PF, half)])

            # ---- output: cast history to fp32 ----
            nc.gpsimd.dma_start(ho_d[:], hist[:, KH * BS:(t_steps + 1) * KH * BS])

    nc.compile()
    return nc


def _pack_inputs(x, h0, c0, W_f_short, b_f_short, W_f_long, b_f_long,
                 W_alpha, b_alpha, W_m, b_m, W_C, b_C, W_o, b_o, t_steps):
    W_all = np.concatenate(
        [W_f_short, W_f_long, W_alpha, W_o, W_m, W_C], axis=1).astype(np.float32)
    b_all = np.concatenate(
        [b_f_short, b_f_long, b_alpha, b_o, b_m, b_C], axis=0).astype(np.float32)
    # Wsb[p, kc*WCOL + m] = W_all[kc*128 + p, m]
    wsb = np.ascontiguousarray(
        W_all.reshape(6, 128, WCOL).transpose(1, 0, 2).reshape(128, 6 * WCOL)
    ).astype(BF16)
    b22 = np.ascontiguousarray(b_all.reshape(MT, 128).T).astype(np.float32)
    bc1 = np.ascontiguousarray(
        np.repeat(b_C.astype(np.float32).reshape(MT_C, 128).T[:, :, None],
                  BS, axis=2).reshape(128, MT_C * BS))
    bc64 = np.tile(bc1, (1, TB)).astype(BF16)
    eye = np.eye(128, dtype=np.float32).astype(BF16)

    ins = []
    for i in range(NC):
        xi = np.asarray(x[i * BS:(i + 1) * BS, :t_steps]).astype(np.float32)
        # xt[p, kc*T*BS + t*BS + b] = x[b, t, kc*128 + p]
        xti = np.ascontiguousarray(
            xi.reshape(BS, t_steps, KX, 128).transpose(3, 2, 1, 0)
            .reshape(128, KX * t_steps * BS)).astype(BF16)
        h0i = np.ascontiguousarray(
            np.asarray(h0[i * BS:(i + 1) * BS]).astype(np.float32)
            .reshape(BS, KH, 128).transpose(2, 1, 0).reshape(128, KH * BS)
        ).astype(BF16)
        c0i = np.ascontiguousarray(
            np.asarray(c0[i * BS:(i + 1) * BS]).astype(np.float32)
            .reshape(BS, MT_C, 128).transpose(2, 1, 0).reshape(128, MT_C * BS)
        ).astype(np.float32)
        ins.append({"wsb": wsb, "xt": xti, "b22": b22, "bc64": bc64,
                    "eye": eye, "h0p": h0i, "c0p": c0i})
    return ins


def kernel(**inputs):
    t_steps = int(np.asarray(inputs["x"]).shape[1])
    if t_steps not in _CACHE:
        _CACHE[t_steps] = _build_program(t_steps)
    nc = _CACHE[t_steps]

    from concourse.bass_utils import run_bass_kernel_spmd
    ins = _pack_inputs(t_steps=t_steps, **inputs)
    res = run_bass_kernel_spmd(nc, ins, core_ids=list(range(NC)))

    out = np.empty((B, t_steps, U), dtype=np.float32)
    for i in range(NC):
        ho = np.asarray(res.results[i]["ho"])  # [128, T*KH*BS]
        a = ho.reshape(128, t_steps, KH, BS)
        out[i * BS:(i + 1) * BS] = a.transpose(3, 1, 2, 0).reshape(BS, t_steps, U)
    return out


if __name__ == "__main__":
    rng = np.random.default_rng(0)
    sh = {"x": (B, T, D), "h0": (B, U), "c0": (B, U)}
    demo = {k: rng.standard_normal(v).astype(np.float32) * 0.1
            for k, v in sh.items()}
    for n, s in [("W_f_short", (D + U, U)), ("W_f_long", (D + U, U)),
                 ("W_alpha", (D + U, U)), ("W_m", (D + U, D)),
                 ("W_C", (D + U, U)), ("W_o", (D + U, U))]:
        demo[n] = rng.standard_normal(s).astype(np.float32) * 0.05
    for n, s in [("b_f_short", U), ("b_f_long", U), ("b_alpha", U),
                 ("b_m", D), ("b_C", U), ("b_o", U)]:
        demo[n] = np.zeros(s, np.float32)
    out = kernel(**demo)
    print(out.shape, out.dtype)
